# revision 1
# baseline (speedup 1.0000x reference)
"""Causal single-head attention forward (B=4, T=4096, C=256, H=64) on 8 NeuronCores.

Sharding: core = (batch, kv_parity).  Each core processes ALL queries of its
batch, but only the kv tiles (128 keys each) whose global tile index has its
parity (even/odd interleave) -- this balances the causal workload exactly
across the two cores of a batch.  Each core emits the *unnormalized* softmax
numerator (exp(S) @ V, transposed: [H, T]) plus the denominator row [1, T];
the host merges the two partials per batch: out = (u0+u1)/(d0+d1), then
transposes back.

On-chip layout is "transposed activation" space: host passes xT = x[b].T so
projections contract C on partitions; scores are computed transposed
(S^T = K Q^T, [keys, q]) so the AV matmul can use V in natural [keys, H]
layout as the stationary operand with keys as the contraction dim.  A column
of ones appended to V folds the softmax denominator into the same matmul.
"""

import sys

for _p in ("/opt/trn_rl_repo", "/root/.axon_site/_ro/trn_rl_repo"):
    if _p not in sys.path:
        sys.path.append(_p)

from contextlib import ExitStack

import numpy as np

import concourse.bacc as bacc
import concourse.bass as bass
import concourse.tile as tile
from concourse import mybir
from concourse.bass_utils import run_bass_kernel_spmd

B, T, C, H = 4, 4096, 256, 64
TK = T // 2      # kv columns owned by one core (its parity's tiles)
QB = 512         # query block width
NQB = T // QB    # 8 query blocks
KT = 128         # kv tile width
F32 = mybir.dt.float32
BF16 = mybir.dt.bfloat16
SCALE = float(C) ** -0.5

_NC = None


def build_nc() -> bass.Bass:
    nc = bacc.Bacc("TRN2", target_bir_lowering=False, debug=False)
    xT = nc.declare_dram_parameter("xT", [C, T], BF16, isOutput=False)
    xkvT = nc.declare_dram_parameter("xkvT", [C, TK], BF16, isOutput=False)
    wq = nc.declare_dram_parameter("wq", [C, H], BF16, isOutput=False)
    wk = nc.declare_dram_parameter("wk", [C, H], BF16, isOutput=False)
    wv = nc.declare_dram_parameter("wv", [C, H], BF16, isOutput=False)
    masks = nc.declare_dram_parameter("masks", [KT, 2 * QB], BF16, isOutput=False)
    uT = nc.declare_dram_parameter("uT", [H, T], F32, isOutput=True)
    den = nc.declare_dram_parameter("den", [1, T], F32, isOutput=True)

    with tile.TileContext(nc) as tc, ExitStack() as ctx:
        persist = ctx.enter_context(tc.tile_pool(name="persist", bufs=1))
        pexp = ctx.enter_context(tc.tile_pool(name="exp", bufs=4))
        pout = ctx.enter_context(tc.tile_pool(name="out", bufs=2))
        pproj = ctx.enter_context(tc.tile_pool(name="pproj", bufs=2, space="PSUM"))
        pqk = ctx.enter_context(tc.tile_pool(name="pqk", bufs=2, space="PSUM"))
        pav = ctx.enter_context(tc.tile_pool(name="pav", bufs=2, space="PSUM"))

        # ---- load weights + masks ------------------------------------------
        w_sb = {}
        for name, dram in (("q", wq), ("k", wk), ("v", wv)):
            for cc in range(2):
                t = persist.tile([128, H], BF16, tag=f"w{name}{cc}")
                nc.sync.dma_start(out=t[:], in_=dram[128 * cc : 128 * (cc + 1), :])
                w_sb[name, cc] = t
        m_sb = persist.tile([KT, 2 * QB], BF16, tag="mask")
        nc.sync.dma_start(out=m_sb[:], in_=masks[:])

        # ---- load xT (full, for Q) and xkvT (gathered, for K/V) ------------
        # j-major emission so early q-blocks' data (and kv data) arrive first
        x_sb = {}
        xkv_sb = {}
        for j in range(NQB):
            for cc in range(2):
                t = persist.tile([128, QB], BF16, tag=f"x{cc}_{j}")
                nc.sync.dma_start(
                    out=t[:], in_=xT[128 * cc : 128 * (cc + 1), QB * j : QB * (j + 1)]
                )
                x_sb[cc, j] = t
            if j < TK // QB:
                for cc in range(2):
                    t = persist.tile([128, QB], BF16, tag=f"xk{cc}_{j}")
                    nc.sync.dma_start(
                        out=t[:],
                        in_=xkvT[128 * cc : 128 * (cc + 1), QB * j : QB * (j + 1)],
                    )
                    xkv_sb[cc, j] = t

        # ---- projections ----------------------------------------------------
        # qT[64, T] in 8 blocks; kT[64, TK] in 4 blocks; contract C in 2 chunks
        q_sb = [None] * NQB
        k_sb = [None] * (TK // QB)
        v_sb = [None] * (TK // KT)

        def proj_q(j):
            ps = pproj.tile([64, QB], F32, tag="proj")
            for cc in range(2):
                nc.tensor.matmul(
                    ps[:], lhsT=w_sb["q", cc][:], rhs=x_sb[cc, j][:],
                    start=(cc == 0), stop=(cc == 1),
                )
            t = persist.tile([64, QB], BF16, tag=f"qT{j}")
            nc.vector.tensor_copy(t[:], ps[:])
            q_sb[j] = t

        def proj_k(j):
            ps = pproj.tile([64, QB], F32, tag="proj")
            for cc in range(2):
                nc.tensor.matmul(
                    ps[:], lhsT=w_sb["k", cc][:], rhs=xkv_sb[cc, j][:],
                    start=(cc == 0), stop=(cc == 1),
                )
            t = persist.tile([64, QB], BF16, tag=f"kT{j}")
            nc.vector.tensor_copy(t[:], ps[:])
            k_sb[j] = t

        def proj_v(tt):
            # v natural [keys, H] + ones column (denominator fold):
            # lhsT = xkvT chunk (stationary), rhs = Wv chunk
            ps = pproj.tile([128, H], F32, tag="proj")
            j, o = divmod(tt, 4)
            for cc in range(2):
                nc.tensor.matmul(
                    ps[:],
                    lhsT=xkv_sb[cc, j][:, KT * o : KT * (o + 1)],
                    rhs=w_sb["v", cc][:],
                    start=(cc == 0), stop=(cc == 1),
                )
            t = persist.tile([128, H + 1], BF16, tag=f"v{tt}")
            nc.vector.tensor_copy(t[:, 0:H], ps[:])
            nc.vector.memset(t[:, H : H + 1], 1.0)
            v_sb[tt] = t

        # interleave so q-block 0's dependencies are produced first
        for j in range(TK // QB):
            proj_k(j)
            for tt in range(4 * j, 4 * j + 4):
                proj_v(tt)
            proj_q(2 * j)
            proj_q(2 * j + 1)

        # ---- attention ------------------------------------------------------
        # kv tiles in pairs: one [128, 2*QB] psum (2 banks) per pair, a single
        # exp over both; AV matmuls run one pair BEHIND the QK/exp stream so
        # the in-order PE never blocks on ACT's current exp.
        for p in range(NQB):
            ns = 2 * (p + 1)  # local kv tiles visible to this q block
            npair = ns // 2
            av = pav.tile([H + 1, QB], F32, tag="av")

            def av_pair(P, exP, last):
                for half in range(2):
                    s = 2 * P + half
                    nc.tensor.matmul(
                        av[:], lhsT=v_sb[s][:],
                        rhs=exP[:, QB * half : QB * (half + 1)],
                        start=(s == 0), stop=(last and half == 1),
                    )

            exs = []
            for P in range(npair):
                qk2 = pqk.tile([KT, 2 * QB], F32, tag="qk")
                for half in range(2):
                    s = 2 * P + half
                    j, o = divmod(s, 4)
                    nc.tensor.matmul(
                        qk2[:, QB * half : QB * (half + 1)],
                        lhsT=k_sb[j][:, KT * o : KT * (o + 1)],
                        rhs=q_sb[p][:],
                        start=True, stop=True,
                    )
                ex = pexp.tile([KT, 2 * QB], BF16, tag="exp")
                nc.scalar.activation(
                    ex[:], qk2[:], mybir.ActivationFunctionType.Exp, scale=SCALE
                )
                if P == npair - 1:
                    nc.vector.tensor_mul(ex[:], ex[:], m_sb[:])
                exs.append(ex)
                if P >= 1:
                    av_pair(P - 1, exs[P - 1], last=False)
            av_pair(npair - 1, exs[npair - 1], last=True)

            ot = pout.tile([H + 1, QB], F32, tag="out")
            nc.vector.tensor_copy(ot[:], av[:])
            nc.sync.dma_start(out=uT[:, QB * p : QB * (p + 1)], in_=ot[0:H, :])
            nc.sync.dma_start(out=den[:, QB * p : QB * (p + 1)], in_=ot[H : H + 1, :])

    nc.compile()
    return nc


def get_nc() -> bass.Bass:
    global _NC
    if _NC is None:
        _NC = build_nc()
    return _NC


def make_in_maps(x, Wk, Wq, Wv):
    import ml_dtypes

    bf16 = ml_dtypes.bfloat16
    x = np.ascontiguousarray(np.asarray(x, np.float32).astype(bf16))
    Wk = np.ascontiguousarray(np.asarray(Wk, np.float32).astype(bf16))
    Wq = np.ascontiguousarray(np.asarray(Wq, np.float32).astype(bf16))
    Wv = np.ascontiguousarray(np.asarray(Wv, np.float32).astype(bf16))
    jj = np.arange(QB)[None, :]
    kk = np.arange(KT)[:, None]
    M = [(jj >= kk + KT * d).astype(bf16) for d in range(4)]
    in_maps = []
    for core in range(8):
        b, par = divmod(core, 2)
        xTb = np.ascontiguousarray(x[b].T)  # [C, T]
        # gather this parity's kv tiles: global tile g = 2s+par -> local slot s
        cols = (
            (2 * np.arange(TK // KT)[:, None] + par) * KT + np.arange(KT)[None, :]
        ).reshape(-1)
        xkvT = np.ascontiguousarray(xTb[:, cols])
        in_maps.append(
            {
                "xT": xTb,
                "xkvT": xkvT,
                "wq": Wq,
                "wk": Wk,
                "wv": Wv,
                "masks": np.ascontiguousarray(
                    np.concatenate([M[par], M[par + 2]], axis=1)
                ),
            }
        )
    return in_maps


def merge(results):
    out = np.empty((B, T, H), np.float32)
    for b in range(B):
        num = results[2 * b]["uT"] + results[2 * b + 1]["uT"]  # [H, T]
        d = results[2 * b]["den"] + results[2 * b + 1]["den"]  # [1, T]
        out[b] = (num / d).T
    return out


def kernel(x, Wk, Wq, Wv, **kw):
    in_maps = make_in_maps(x, Wk, Wq, Wv)
    res = run_bass_kernel_spmd(get_nc(), in_maps, core_ids=list(range(8)), **kw)
    out = merge(res.results)
    if kw:
        return out, res
    return out



# revision 11
# speedup vs baseline: 1.0413x; 1.0413x over previous
"""Causal single-head attention forward (B=4, T=4096, C=256, H=64) on 8 NeuronCores.

Sharding: core = (batch, kv_parity).  Each core processes ALL queries of its
batch but only kv tiles (128 keys) whose global tile index has its parity
(even/odd interleave), which balances the causal workload across the two
cores of a batch.  Each core emits unnormalized numerator+denominator stacked
as ud[65, T] (rows 0:64 = (exp(S)@V)^T, row 64 = sum exp(S)); the host
merges: out = (u0+u1)/(d0+d1), transposed back.

The compiled program is parity-uniform; parity only enters through host-
prepared data (gathered xkv columns and the diagonal-pair mask values).

Engine plan per core:
- PE (bf16): q projections from full x, k/v projections from the gathered
  parity columns, S^T = K Q^T per 128-key tile, AV with V stationary (ones
  column folds the denominator).  The diagonal pair's second tile is
  col-trimmed to [256, 512) (parity-uniform superset of the causal region).
  AV lags the QK/exp stream by 2 pairs so the PE queue never drains (keeps
  the tensor engine's p-state ramped).
- ACT: exact exp (fp32 psum -> bf16) for the diagonal pair + even non-diag
  pairs; table pre-warmed during input DMA.
- DVE: exp for odd non-diag pairs via one tensor_scalar emitting bf16 BITS
  as int16 (Schraudolph: bits = round(s*A + B), ~+-3% per-element, max-norm
  safe), q/k psum->bf16 casts, diagonal mask multiplies.
- Pool (gpsimd): v psum->bf16 copies, ones-column memsets, av psum->sbuf
  output copies.
"""

import sys

for _p in ("/opt/trn_rl_repo", "/root/.axon_site/_ro/trn_rl_repo"):
    if _p not in sys.path:
        sys.path.append(_p)

from contextlib import ExitStack

import numpy as np

import concourse.bacc as bacc
import concourse.bass as bass
import concourse.tile as tile
from concourse import mybir
from concourse.bass_utils import run_bass_kernel_spmd

B, T, C, H = 4, 4096, 256, 64
QB = 512         # query block width
NQB = T // QB    # 8 query blocks
KT = 128         # kv tile width
TK = T // 2      # gathered kv columns per core
F32 = mybir.dt.float32
BF16 = mybir.dt.bfloat16
I16 = mybir.dt.int16
SCALE = float(C) ** -0.5
# Schraudolph bf16: bits = round(s*A + B) ~ exp(s*SCALE)
SCH_A = 128.0 / float(np.log(2.0)) * SCALE
SCH_B = 16248.65
O1 = 256         # uniform col-trim offset for the diagonal pair's 2nd tile

_NC = None


def build_nc() -> bass.Bass:
    nc = bacc.Bacc("TRN2", target_bir_lowering=False, debug=False)
    xq = nc.declare_dram_parameter("xq", [128, 2, T], BF16, isOutput=False)
    xkv = nc.declare_dram_parameter("xkv", [128, 2, TK], BF16, isOutput=False)
    wq = nc.declare_dram_parameter("wq", [128, 2, H], BF16, isOutput=False)
    wk = nc.declare_dram_parameter("wk", [128, 2, H], BF16, isOutput=False)
    wv = nc.declare_dram_parameter("wv", [128, 2, H], BF16, isOutput=False)
    msk = nc.declare_dram_parameter("msk", [KT, QB], BF16, isOutput=False)
    ud = nc.declare_dram_parameter("ud", [H + 1, T], F32, isOutput=True)

    with tile.TileContext(nc) as tc, ExitStack() as ctx:
        persist = ctx.enter_context(tc.tile_pool(name="persist", bufs=1))
        pexp = ctx.enter_context(tc.tile_pool(name="exp", bufs=4))
        pout = ctx.enter_context(tc.tile_pool(name="out", bufs=2))
        pproj = ctx.enter_context(tc.tile_pool(name="pproj", bufs=2, space="PSUM"))
        pqk = ctx.enter_context(tc.tile_pool(name="pqk", bufs=2, space="PSUM"))
        pav = ctx.enter_context(tc.tile_pool(name="pav", bufs=2, space="PSUM"))

        # ---- warm the ACT exp table during input DMA -------------------------
        warm = persist.tile([1, 2], F32, tag="warm")
        nc.vector.memset(warm[:], 0.0)
        nc.scalar.activation(warm[:], warm[:], mybir.ActivationFunctionType.Exp)

        # ---- load weights + mask + x ----------------------------------------
        w_sb = {}
        for name, dram in (("q", wq), ("k", wk), ("v", wv)):
            t = persist.tile([128, 2, H], BF16, tag=f"w{name}")
            nc.sync.dma_start(out=t[:], in_=dram[:])
            w_sb[name] = t
        m_sb = persist.tile([KT, QB], BF16, tag="mask")
        nc.sync.dma_start(out=m_sb[:], in_=msk[:])
        xkv_sb = persist.tile([128, 2, TK], BF16, tag="xkv")
        for j in range(TK // QB):
            nc.sync.dma_start(
                out=xkv_sb[:, :, QB * j : QB * (j + 1)],
                in_=xkv[:, :, QB * j : QB * (j + 1)],
            )
        xq_sb = persist.tile([128, 2, T], BF16, tag="xq")
        for j in range(NQB):
            nc.sync.dma_start(
                out=xq_sb[:, :, QB * j : QB * (j + 1)],
                in_=xq[:, :, QB * j : QB * (j + 1)],
            )

        # ---- projections (bf16, contract C in 2 chunks) ---------------------
        q_sb = [None] * NQB           # bf16 [64, QB]
        k_sb = [None] * (TK // QB)    # bf16 [64, QB] local gathered layout
        v_sb = [None] * NQB           # bf16 [128, 2, 65] per pair

        def proj_qk(which, src, j, dst_list):
            ps = pproj.tile([64, QB], F32, tag="proj")
            for c in range(2):
                nc.tensor.matmul(
                    ps[:], lhsT=w_sb[which][:, c, :],
                    rhs=src[:, c, QB * j : QB * (j + 1)],
                    start=(c == 0), stop=(c == 1),
                )
            t = persist.tile([64, QB], BF16, tag=f"{which}{j}")
            nc.vector.tensor_copy(t[:], ps[:])
            dst_list[j] = t

        def proj_v(P):
            # pair P covers local kv tiles 2P, 2P+1 -> gathered cols 128s
            ps = pproj.tile([128, 2, H], F32, tag="proj")
            for h in range(2):
                s = 2 * P + h
                for c in range(2):
                    nc.tensor.matmul(
                        ps[:, h, :],
                        lhsT=xkv_sb[:, c, KT * s : KT * (s + 1)],
                        rhs=w_sb["v"][:, c, :],
                        start=(c == 0), stop=(c == 1),
                    )
            t = persist.tile([128, 2, H + 1], BF16, tag=f"v{P}")
            nc.vector.tensor_copy(t[:, :, 0:H], ps[:])
            nc.gpsimd.memset(t[:, :, H : H + 1], 1.0)
            v_sb[P] = t

        # k/v first (block 0's deps), then interleave q
        for j in range(TK // QB):
            proj_qk("k", xkv_sb, j, k_sb)
            proj_v(2 * j)
            proj_v(2 * j + 1)
            proj_qk("q", xq_sb, 2 * j, q_sb)
            proj_qk("q", xq_sb, 2 * j + 1, q_sb)

        # ---- attention -------------------------------------------------------
        def k_slice(s):  # local kv tile s -> gathered k columns
            return k_sb[s // 4][:, KT * (s % 4) : KT * (s % 4 + 1)]

        nslot = [0]

        def emit_qk_exp(p, P):
            diag = P == p
            if not diag:
                nslot[0] += 1
            qk2 = pqk.tile([KT, 2 * QB], F32, tag="qk")
            nc.tensor.matmul(
                qk2[:, 0:QB], lhsT=k_slice(2 * P), rhs=q_sb[p][:],
                start=True, stop=True,
            )
            if diag:
                nc.tensor.matmul(
                    qk2[:, QB + O1 : 2 * QB], lhsT=k_slice(2 * P + 1),
                    rhs=q_sb[p][:, O1:QB], start=True, stop=True,
                )
            else:
                nc.tensor.matmul(
                    qk2[:, QB : 2 * QB], lhsT=k_slice(2 * P + 1), rhs=q_sb[p][:],
                    start=True, stop=True,
                )
            ex = pexp.tile([KT, 2 * QB], BF16, tag="exp")
            if diag:
                nc.scalar.activation(
                    ex[:, 0:QB], qk2[:, 0:QB],
                    mybir.ActivationFunctionType.Exp, scale=SCALE,
                )
                nc.scalar.activation(
                    ex[:, QB + O1 : 2 * QB], qk2[:, QB + O1 : 2 * QB],
                    mybir.ActivationFunctionType.Exp, scale=SCALE,
                )
                # masks: region h0 = ex[:, 0:256] (*= msk[:, 0:256]),
                #        region h1 = ex[:, 768:1024] (*= msk[:, 256:512])
                nc.gpsimd.tensor_mul(ex[:, 0:O1], ex[:, 0:O1], m_sb[:, 0:O1])
                nc.gpsimd.tensor_mul(
                    ex[:, QB + O1 : 2 * QB], ex[:, QB + O1 : 2 * QB],
                    m_sb[:, O1:QB],
                )
            elif nslot[0] % 3 != 0:
                nc.scalar.activation(
                    ex[:], qk2[:], mybir.ActivationFunctionType.Exp, scale=SCALE
                )
            else:
                nc.vector.tensor_scalar(
                    ex[:].bitcast(I16), qk2[:], SCH_A, SCH_B,
                    mybir.AluOpType.mult, mybir.AluOpType.add,
                )
            return ex

        av_tiles = {}

        def emit_av(p, P, ex):
            diag = P == p
            av = av_tiles[p]
            nc.tensor.matmul(
                av[:], lhsT=v_sb[P][:, 0, :], rhs=ex[:, 0:QB],
                start=(P == 0), stop=False,
            )
            if diag:
                nc.tensor.matmul(
                    av[:, O1:QB], lhsT=v_sb[P][:, 1, :],
                    rhs=ex[:, QB + O1 : 2 * QB], start=False, stop=True,
                )
            else:
                nc.tensor.matmul(
                    av[:, 0:QB], lhsT=v_sb[P][:, 1, :], rhs=ex[:, QB : 2 * QB],
                    start=False, stop=False,
                )
            if diag:  # block finished: drain, DMA out
                ot = pout.tile([H + 1, QB], F32, tag="out")
                nc.vector.tensor_copy(ot[:], av[:])
                nc.sync.dma_start(out=ud[:, QB * p : QB * (p + 1)], in_=ot[:])

        pending = []
        for p in range(NQB):
            av = pav.tile([H + 1, QB], F32, tag="av")
            av_tiles[p] = av
            for P in range(p + 1):
                ex = emit_qk_exp(p, P)
                pending.append((p, P, ex))
                if len(pending) > 2:
                    emit_av(*pending.pop(0))
        while pending:
            emit_av(*pending.pop(0))

    nc.compile()
    return nc


def get_nc() -> bass.Bass:
    global _NC
    if _NC is None:
        _NC = build_nc()
    return _NC


def make_in_maps(x, Wk, Wq, Wv):
    import ml_dtypes

    bf16 = ml_dtypes.bfloat16
    x = np.asarray(x, np.float32)

    def wpack(W):
        return np.ascontiguousarray(
            np.asarray(W, np.float32).reshape(2, 128, H).transpose(1, 0, 2)
        ).astype(bf16)

    wq8, wk8, wv8 = wpack(Wq), wpack(Wk), wpack(Wv)

    kk = np.arange(KT)[:, None]
    jj = np.arange(QB)[None, :]
    in_maps = []
    for core in range(8):
        b, par = divmod(core, 2)
        xb = x[b].T.reshape(2, 128, T).transpose(1, 0, 2)  # [128, 2, T]
        xq = np.ascontiguousarray(xb).astype(bf16)
        # gathered parity columns: local tile s -> global tile g=2s+par
        cols = (
            (2 * np.arange(TK // KT)[:, None] + par) * KT + np.arange(KT)[None, :]
        ).reshape(-1)
        xkv = np.ascontiguousarray(xb[:, :, cols]).astype(bf16)
        # mask [128, 512]: cols 0:256 for diag tile d0 (offset 128*par),
        # cols 256:512 for diag tile d1 (offset 256+128*par), both relative
        # to the computed regions (h0 cols 0:256 of q-block, h1 cols 256:512).
        m = np.zeros((KT, QB), np.float32)
        m[:, 0:O1] = (jj[:, 0:O1] >= kk + 128 * par).astype(np.float32)
        m[:, O1:QB] = (jj[:, O1:QB] >= kk + O1 + 128 * par).astype(np.float32)
        in_maps.append(
            {"xq": xq, "xkv": xkv, "wq": wq8, "wk": wk8, "wv": wv8,
             "msk": m.astype(bf16)}
        )
    return in_maps


def merge(results):
    out = np.empty((B, T, H), np.float32)
    for b in range(B):
        s = results[2 * b]["ud"] + results[2 * b + 1]["ud"]  # [65, T]
        out[b] = (s[0:H] / s[H : H + 1]).T
    return out


def kernel(x, Wk, Wq, Wv, **kw):
    in_maps = make_in_maps(x, Wk, Wq, Wv)
    res = run_bass_kernel_spmd(get_nc(), in_maps, core_ids=list(range(8)), **kw)
    out = merge(res.results)
    if kw:
        return out, res
    return out
